# revision 1
# baseline (speedup 1.0000x reference)
"""Hashed-weight MLP (1024-4096-4096-32000, batch 2048) on 8 TRN2 NeuronCores.

Problem: h = relu(x @ W0); h = relu(h @ W1); out = h @ W2, where each
W_l[i, j] = hw_l[(a_l*i + b_l*j + c_l) % N_l] is a virtual (ROBE-Z hashed)
weight gathered from a small parameter vector.

Approach (column-parallel tensor parallelism on all three layers):
  * Since N_l is a power of two and b_l is odd, b_l is invertible mod N_l.
    Through the host-computed permuted table hw_bb[t] = hw[(b*t) % N], the
    virtual weight becomes ROW-CONTIGUOUS:
        W[i, j0+dj] = hw_bb[u_i + j0 + dj],   u_i = b^-1 (a*i + c) % N
    and row starts u_i form an arithmetic progression mod N with stride
    q = b^-1 a. A whole [in_dim x width] weight shard therefore materializes
    with a handful of 3-level strided DMAs (signed-residue ladder over q) —
    NO per-element gathers anywhere.
  * Each core owns a 1/8 column shard of every layer. Per-core shard offsets
    are absorbed into the host-side slice of hw_bb (keeping the device
    program SPMD-uniform). Activations stay transposed [features, batch].
  * GEMMs are bf16 with fp32 PSUM accumulation (max-rel-err ~4e-3).
    AllGathers after L0/L1 are chunked per batch-tile and hidden behind the
    next batch-tile's matmuls; weight materializations are pipelined on the
    scalar/sync HWDGE rings under compute.
"""
import sys
if "/opt/trn_rl_repo" not in sys.path:
    sys.path.insert(0, "/opt/trn_rl_repo")

import numpy as np
import ml_dtypes

import concourse.bass as bass
import concourse.bacc as bacc
import concourse.tile as tile
import concourse.mybir as mybir
from concourse.bass_utils import run_bass_kernel_spmd

N_CORES = 8
P = 128
NB = 512                      # moving free dim (batch tile)
BATCH = 2048
BT = BATCH // NB              # 4

LENS = [1024, 4096, 4096, 32000]
HASH_A = [9973, 10007, 10039]
HASH_B = [31013, 31019, 31039]
HASH_C = [557, 563, 569]
SIZES = [1048576, 1048576, 4194304]

JW = [512, 512, 4000]         # true per-core output shard width
WMAT = [512, 512, 4096]       # materialized width (L2 padded to 32 j-tiles)

BF = mybir.dt.bfloat16
F32 = mybir.dt.float32


def _plan_layer(l):
    N = SIZES[l]; a, b, ch = HASH_A[l], HASH_B[l], HASH_C[l]
    binv = pow(b, -1, N)
    q = (binv * a) % N
    u0 = (binv * ch) % N
    in_dim = LENS[l]; w = WMAT[l]
    best = None
    for k in range(1, min(in_dim, 600) + 1):
        r = (q * k) % N
        if r > N // 2:
            r -= N
        C1 = -(-in_dim // k)
        extra = q * (k - 1) + abs(r) * (C1 - 1)
        if best is None or extra < best[0]:
            best = (extra, k, C1, r)
    _, k, C1, r = best
    shift = max(0, -r * (C1 - 1))
    m_ext = shift + q * (k - 1) + max(r, 0) * (C1 - 1) + w + 64
    if l == 1:
        # L1 is materialized in two row-halves with an extra mod-N base
        # offset on the second half; cover it in the slice.
        m_ext += N
    return dict(N=N, a=a, b=b, ch=ch, q=q, u0=u0, k=k, C1=C1, r=r,
                shift=shift, m_ext=m_ext, rows=k * C1, in_dim=in_dim, w=w)


PLANS = [_plan_layer(l) for l in range(3)]
RG = [list(range(N_CORES))]


def build_nc():
    nc = bacc.Bacc("TRN2", target_bir_lowering=False, debug=False,
                   num_devices=N_CORES)

    xT_d = nc.dram_tensor("xT", [LENS[0], BATCH], BF, kind="ExternalInput").ap()
    hb = [nc.dram_tensor(f"hb{l}", [PLANS[l]["m_ext"]], BF,
                         kind="ExternalInput").ap() for l in range(3)]
    w_mat = [nc.dram_tensor(f"w{l}", [PLANS[l]["rows"], PLANS[l]["w"]], BF).ap()
             for l in range(1)]
    # W1 in two row-half tensors so layer 1 starts after half the
    # materialization; halves are driven from both HWDGE rings in parallel.
    HROWS = 43 * 48           # 2064 >= 2048 rows per half (k=43, C1_half=48)
    w1_h = [nc.dram_tensor(f"w1h{h}", [HROWS, 512], BF).ap() for h in range(2)]
    # L2 weight shard split into 4 j-group tensors so L2 can start as soon as
    # its first slab is materialized (dep tracking is per-tensor).
    w2_jg = [nc.dram_tensor(f"w2jg{g}", [PLANS[2]["rows"], 1024], BF).ap()
             for g in range(4)]
    # per-batch-tile activation chunks: local contribution + allgathered full
    h1c = [nc.dram_tensor(f"h1c{b}", [512, NB], BF).ap() for b in range(BT)]
    h1f = [nc.dram_tensor(f"h1f{b}", [4096, NB], BF, addr_space="Shared").ap()
           for b in range(BT)]
    h2c = [nc.dram_tensor(f"h2c{b}", [512, NB], BF).ap() for b in range(BT)]
    h2f = [nc.dram_tensor(f"h2f{b}", [4096, NB], BF, addr_space="Shared").ap()
           for b in range(BT)]
    out_d = nc.dram_tensor("outT", [4096, BATCH], F32, kind="ExternalOutput").ap()

    def matz_small(l):
        """Materialize W0. dim0 = a <=16-count block of the i1 axis: walrus
        splits a DMA across SDMA engine slots by the outermost dim only when
        its count is <= 16, so this shape fans out 16-wide."""
        pl = PLANS[l]
        q, k, C1, r, w = pl["q"], pl["k"], pl["C1"], pl["r"], pl["w"]
        off = pl["shift"]
        with nc.allow_non_contiguous_dma(reason="hash ladder materialization"):
            for k0 in range(0, k, 16):
                kc = min(16, k - k0)
                src = bass.AP(hb[l].tensor, off + q * k0,
                              [[q, kc], [r, C1], [1, w]])
                dst = bass.AP(w_mat[l].tensor, w * k0,
                              [[w, kc], [k * w, C1], [1, w]])
                nc.scalar.dma_start(out=dst, in_=src)

    def matz1_half(h):
        """Materialize W1 rows [2048h, 2048h+2064) from the periodic slice.
        Row i = 2048h + i0 + 43*i1; base offset (q*2048h) mod N."""
        pl = PLANS[1]
        q, k, r, w, N = pl["q"], pl["k"], pl["r"], pl["w"], pl["N"]
        C1h = 48
        off = pl["shift"] + (q * 2048 * h) % N
        eng = nc.scalar if h == 0 else nc.sync
        with nc.allow_non_contiguous_dma(reason="hash ladder materialization"):
            for k0 in range(0, k, 16):
                kc = min(16, k - k0)
                src = bass.AP(hb[1].tensor, off + q * k0,
                              [[q, kc], [r, C1h], [1, w]])
                dst = bass.AP(w1_h[h].tensor, w * k0,
                              [[w, kc], [k * w, C1h], [1, w]])
                eng.dma_start(out=dst, in_=src)

    def matz2(jgs):
        """Materialize L2 j-group slabs (2KB inner runs spread well)."""
        pl = PLANS[2]
        q, k, C1, r = pl["q"], pl["k"], pl["C1"], pl["r"]
        w = 1024
        nchunk = 4
        step = -(-C1 // nchunk)
        with nc.allow_non_contiguous_dma(reason="hash ladder materialization"):
            for g in jgs:
                for ci in range(nchunk):
                    c1a = ci * step
                    c1b = min(C1, c1a + step)
                    cnt = c1b - c1a
                    src = bass.AP(hb[2].tensor,
                                  pl["shift"] + g * 1024 + r * c1a,
                                  [[q, k], [r, cnt], [1, w]])
                    dst = bass.AP(w2_jg[g].tensor, k * w * c1a,
                                  [[w, k], [k * w, cnt], [1, w]])
                    nc.scalar.dma_start(out=dst, in_=src)

    with tile.TileContext(nc) as tc, \
         tc.tile_pool(name="ps", bufs=8, space="PSUM") as psp, \
         tc.tile_pool(name="slabA", bufs=1) as slabA, \
         tc.tile_pool(name="slabB", bufs=1) as slabB:
        # ---- L0 weight materialization + loads first: compute starts ASAP
        matz_small(0)

        with tc.tile_pool(name="l0", bufs=1) as l0p:
            xsb = [l0p.tile([P, BATCH], BF, name=f"xsb{kt}") for kt in range(8)]
            w0sb = [l0p.tile([P, 512], BF, name=f"w0sb{kt}") for kt in range(8)]
            h1sb = [l0p.tile([P, BATCH], BF, name=f"h1sb{j}") for j in range(4)]
            for kt in range(8):
                nc.sync.dma_start(out=xsb[kt][:], in_=xT_d[kt * P:(kt + 1) * P, :])
                nc.sync.dma_start(out=w0sb[kt][:], in_=w_mat[0][kt * P:(kt + 1) * P, :])

            # L1 materialization overlaps L0 compute (both HWDGE rings),
            # and L2's first j-group follows on the scalar ring.
            matz1_half(0)
            matz1_half(1)
            matz2([0])

            for b in range(BT):
                for j in range(4):
                    ps = psp.tile([P, NB], F32, tag="ps", name=f"ps0_{b}_{j}")
                    for kt in range(8):
                        nc.tensor.matmul(
                            out=ps[:],
                            lhsT=w0sb[kt][:, j * P:(j + 1) * P],
                            rhs=xsb[kt][:, b * NB:(b + 1) * NB],
                            start=(kt == 0), stop=(kt == 7))
                    nc.scalar.activation(out=h1sb[j][:, b * NB:(b + 1) * NB],
                                         in_=ps[:],
                                         func=mybir.ActivationFunctionType.Relu)
                for j in range(4):
                    nc.sync.dma_start(out=h1c[b][j * P:(j + 1) * P, :],
                                      in_=h1sb[j][:, b * NB:(b + 1) * NB])
                # chunked AllGather: hides behind the next batch-tile's matmuls
                nc.gpsimd.collective_compute(
                    "AllGather", mybir.AluOpType.bypass, replica_groups=RG,
                    ins=[h1c[b].opt()], outs=[h1f[b].opt()])

        # ---- Layer 1 ----
        with tc.tile_pool(name="l1w", bufs=1) as l1wp, \
             tc.tile_pool(name="l1r", bufs=6) as l1rp:
            w1sb = [l1wp.tile([P, 512], BF, name=f"w1sb{kt}") for kt in range(32)]
            h2sb = [l1wp.tile([P, NB], BF, name=f"h2sb{j}") for j in range(4)]
            for kt in range(32):
                h, lk = (0, kt) if kt < 16 else (1, kt - 16)
                nc.sync.dma_start(out=w1sb[kt][:],
                                  in_=w1_h[h][lk * P:(lk + 1) * P, :])

            # remaining L2 materialization overlaps L1 compute and AllGathers
            matz2([1, 2, 3])

            # prefetch L2 j-group 0 slab during layer 1
            slab0 = [slabA.tile([P, 1024], BF, tag=f"w2slab{kt}",
                                name=f"w2s_0_{kt}") for kt in range(32)]
            for kt in range(32):
                nc.scalar.dma_start(out=slab0[kt][:],
                                    in_=w2_jg[0][kt * P:(kt + 1) * P, :])

            for b in range(BT):
                pss = [psp.tile([P, NB], F32, tag="ps", name=f"ps1_{b}_{j}")
                       for j in range(4)]
                for kt in range(32):
                    rhs = l1rp.tile([P, NB], BF, tag="l1rhs", name=f"l1r_{b}_{kt}")
                    nc.sync.dma_start(out=rhs[:],
                                      in_=h1f[b][kt * P:(kt + 1) * P, :])
                    for j in range(4):
                        nc.tensor.matmul(
                            out=pss[j][:],
                            lhsT=w1sb[kt][:, j * P:(j + 1) * P],
                            rhs=rhs[:],
                            start=(kt == 0), stop=(kt == 31))
                for j in range(4):
                    nc.scalar.activation(out=h2sb[j][:],
                                         in_=pss[j][:],
                                         func=mybir.ActivationFunctionType.Relu)
                    nc.sync.dma_start(out=h2c[b][j * P:(j + 1) * P, :],
                                      in_=h2sb[j][:])
                nc.gpsimd.collective_compute(
                    "AllGather", mybir.AluOpType.bypass, replica_groups=RG,
                    ins=[h2c[b].opt()], outs=[h2f[b].opt()])

        # ---- Layer 2 (W2 slabbed by j-group, h2f streamed) ----
        with tc.tile_pool(name="l2r", bufs=6) as l2rp, \
             tc.tile_pool(name="l2o", bufs=4) as l2op:
            for jg in range(4):
                if jg == 0:
                    slab = slab0
                else:
                    pool = slabA if jg % 2 == 0 else slabB
                    slab = [pool.tile([P, 1024], BF, tag=f"w2slab{kt}",
                                      name=f"w2s_{jg}_{kt}") for kt in range(32)]
                    for kt in range(32):
                        nc.scalar.dma_start(
                            out=slab[kt][:],
                            in_=w2_jg[jg][kt * P:(kt + 1) * P, :])
                for b in range(BT):
                    pss = [psp.tile([P, NB], F32, tag="ps",
                                    name=f"ps2_{jg}_{b}_{j}") for j in range(8)]
                    for kt in range(32):
                        rhs = l2rp.tile([P, NB], BF, tag="l2rhs",
                                        name=f"l2r_{jg}_{b}_{kt}")
                        nc.sync.dma_start(out=rhs[:],
                                          in_=h2f[b][kt * P:(kt + 1) * P, :])
                        for j in range(8):
                            nc.tensor.matmul(
                                out=pss[j][:],
                                lhsT=slab[kt][:, j * P:(j + 1) * P],
                                rhs=rhs[:],
                                start=(kt == 0), stop=(kt == 31))
                    for j in range(8):
                        osb = l2op.tile([P, NB], F32, tag="l2out",
                                        name=f"l2o_{jg}_{b}_{j}")
                        nc.vector.tensor_copy(out=osb[:], in_=pss[j][:])
                        nc.scalar.dma_start(
                            out=out_d[(jg * 8 + j) * P:(jg * 8 + j + 1) * P,
                                      b * NB:(b + 1) * NB],
                            in_=osb[:])

    nc.compile()
    return nc


_NC_CACHE = None


def _get_nc():
    global _NC_CACHE
    if _NC_CACHE is None:
        _NC_CACHE = build_nc()
    return _NC_CACHE


def _prep_inputs(x, hw0, hw1, hw2):
    """Host prep: transpose x, build per-core periodic permuted-table slices."""
    x = np.asarray(x, np.float32)
    hws = [np.asarray(hw0, np.float32), np.asarray(hw1, np.float32),
           np.asarray(hw2, np.float32)]
    xT = np.ascontiguousarray(x.T).astype(ml_dtypes.bfloat16)

    per_core_hb = [[None] * 3 for _ in range(N_CORES)]
    for l in range(3):
        pl = PLANS[l]
        N, b = pl["N"], pl["b"]
        m_ext = pl["m_ext"]
        jw = JW[l]
        t0 = pl["u0"] - pl["shift"]          # core-0 slice start (in t-space)
        span = m_ext + (N_CORES - 1) * jw
        t = t0 + np.arange(span, dtype=np.int64)
        shared = hws[l][(b * t) % N].astype(ml_dtypes.bfloat16)
        for c in range(N_CORES):
            per_core_hb[c][l] = shared[c * jw: c * jw + m_ext]
    in_maps = []
    for c in range(N_CORES):
        in_maps.append({
            "xT": xT,
            "hb0": per_core_hb[c][0],
            "hb1": per_core_hb[c][1],
            "hb2": per_core_hb[c][2],
        })
    return in_maps


def kernel(x, hw0, hw1, hw2, trace=False):
    nc = _get_nc()
    in_maps = _prep_inputs(x, hw0, hw1, hw2)
    res = run_bass_kernel_spmd(nc, in_maps, list(range(N_CORES)), trace=trace)
    outs = [res.results[c]["outT"][:JW[2], :] for c in range(N_CORES)]
    full = np.concatenate(outs, axis=0)        # [32000, 2048]
    out = np.ascontiguousarray(full.T)         # [2048, 32000] fp32
    kernel.last_results = res
    return out



# revision 3
# speedup vs baseline: 1.0670x; 1.0670x over previous
"""Hashed-weight MLP (1024-4096-4096-32000, batch 2048) on 8 TRN2 NeuronCores.

Problem: h = relu(x @ W0); h = relu(h @ W1); out = h @ W2, where each
W_l[i, j] = hw_l[(a_l*i + b_l*j + c_l) % N_l] is a virtual (ROBE-Z hashed)
weight gathered from a small parameter vector.

Approach (column-parallel tensor parallelism on all three layers):
  * Via the host-permuted table hw_bb[t] = hw[(b*t) % N] the virtual weight
    becomes row-contiguous: W[i, col] = hw_bb[shift + q*kk + r*c1 + col] with
    i = k*c1 + kk (q = b^-1 a, r = signed residue of q*k mod N). Weight tiles
    are DMAd STRAIGHT from the per-core slice into SBUF (no DRAM
    materialization): one 3-level strided DMA per block-aligned tile for
    L0/L2 (partitions in kk-outer permuted order, with the matching
    permutation applied to the rhs activation tiles), ~4 run DMAs per
    natural-order tile for L1.
  * Each core owns a 1/8 column shard of every layer; shard offsets are
    absorbed into the host-side slice (SPMD-uniform device program).
  * L2 keeps the whole 4096 x 2048 h2 activation resident in SBUF (loaded
    once after each AllGather chunk) and streams W2 tiles through a ring,
    so the big GEMM phase reads each byte from HBM exactly once.
  * Engines: scalar = weight-ladder DMA triggers only; sync = activation
    streams/stores; vector = ReLU + PSUM evacuation (bf16 cast); gpsimd =
    AllGather triggers. GEMMs are bf16 with fp32 PSUM (rel err ~4e-3).
"""
import sys
if "/opt/trn_rl_repo" not in sys.path:
    sys.path.insert(0, "/opt/trn_rl_repo")

import numpy as np
import ml_dtypes

import concourse.bass as bass
import concourse.bacc as bacc
import concourse.tile as tile
import concourse.mybir as mybir
from concourse.bass_utils import run_bass_kernel_spmd

N_CORES = 8
P = 128
NB = 512                      # batch tile (matmul moving dim)
BATCH = 2048
BT = BATCH // NB              # 4

LENS = [1024, 4096, 4096, 32000]
HASH_A = [9973, 10007, 10039]
HASH_B = [31013, 31019, 31039]
HASH_C = [557, 563, 569]
SIZES = [1048576, 1048576, 4194304]
JW = [512, 512, 4000]         # true per-core output shard width
WTOT = [512, 512, 4096]       # max ladder col offset (L2 incl. jg offsets)
TILE_H = [120, 128, 126]      # k-tile heights (L0/L2 block-aligned to k)

BF = mybir.dt.bfloat16
F32 = mybir.dt.float32


def _plan_layer(l):
    N = SIZES[l]; a, b, ch = HASH_A[l], HASH_B[l], HASH_C[l]
    binv = pow(b, -1, N)
    q = (binv * a) % N
    u0 = (binv * ch) % N
    in_dim = LENS[l]
    best = None
    for k in range(1, min(in_dim, 600) + 1):
        r = (q * k) % N
        if r > N // 2:
            r -= N
        C1 = -(-in_dim // k)
        extra = q * (k - 1) + abs(r) * (C1 - 1)
        if best is None or extra < best[0]:
            best = (extra, k, C1, r)
    _, k, C1, r = best
    shift = max(0, -r * (C1 - 1))
    m_ext = shift + q * (k - 1) + max(r, 0) * (C1 - 1) + WTOT[l] + 64
    return dict(N=N, b=b, q=q, u0=u0, k=k, r=r, shift=shift,
                m_ext=m_ext, in_dim=in_dim)


PLANS = [_plan_layer(l) for l in range(3)]
RG = [list(range(N_CORES))]


def _tiles_perm(k, rows_total, tile_h):
    """Block-aligned tiles; seg = (kk0, c1_0, kc, c1c, p0); within a seg the
    DMA streams kk-outer so partition p = p0 + kkrel*c1c + c1rel."""
    tiles = []
    i0 = 0
    while i0 < rows_total:
        h = min(tile_h, rows_total - i0)
        c1_0 = i0 // k
        segs = []
        full_c1 = h // k
        if full_c1:
            segs.append((0, c1_0, k, full_c1, 0))
        rem = h - full_c1 * k
        if rem:
            segs.append((0, c1_0 + full_c1, rem, 1, k * full_c1))
        tiles.append((i0, h, segs))
        i0 += h
    return tiles


def _tiles_nat(k, rows_total, tile_h):
    """Natural-order tiles; each seg is a run within one c1 block."""
    tiles = []
    for i0 in range(0, rows_total, tile_h):
        h = min(tile_h, rows_total - i0)
        segs = []
        row = i0
        while row < i0 + h:
            c1, kk = divmod(row, k)
            cnt = min(k - kk, i0 + h - row)
            segs.append((kk, c1, cnt, 1, row - i0))
            row += cnt
        tiles.append((i0, h, segs))
    return tiles


LT = [
    _tiles_perm(PLANS[0]["k"], 1024, TILE_H[0]),   # 9 tiles (8x120 + 64)
    _tiles_nat(PLANS[1]["k"], 4096, TILE_H[1]),    # 32 tiles x ~4 segs
    _tiles_perm(PLANS[2]["k"], 4096, TILE_H[2]),   # 33 tiles (32x126 + 64)
]
NKT = [len(t) for t in LT]
NJG2 = 8                      # L2 j-groups of width 512 (4 j-tiles of 128)


def _ladder_dmas(nc, eng, hb_t, l, wtile_ap, tile_idx, col0, w):
    """Emit ladder DMAs for weight tile tile_idx of layer l into SBUF tile."""
    pl = PLANS[l]
    q, r, shift = pl["q"], pl["r"], pl["shift"]
    _, h, segs = LT[l][tile_idx]
    for (kk0, c1_0, kc, c1c, p0) in segs:
        base = shift + q * kk0 + r * c1_0 + col0
        if c1c == 1:
            src = bass.AP(hb_t, base, [[q, kc], [1, w]])
        else:
            src = bass.AP(hb_t, base, [[q, kc], [r, c1c], [1, w]])
        eng.dma_start(out=wtile_ap[p0:p0 + kc * c1c, :], in_=src)


def _act_dmas(nc, eng, act_t, l, dst_ap, tile_idx, rowstride, col0, w):
    """Load activation rows matching weight tile tile_idx's partition order.
    act_t is a DRAM tensor whose rows are the layer's contraction rows."""
    pl = PLANS[l]
    k = pl["k"]
    _, h, segs = LT[l][tile_idx]
    for (kk0, c1_0, kc, c1c, p0) in segs:
        base = (k * c1_0 + kk0) * rowstride + col0
        if c1c == 1:
            src = bass.AP(act_t, base, [[rowstride, kc], [1, w]])
        else:
            src = bass.AP(act_t, base,
                          [[rowstride, kc], [k * rowstride, c1c], [1, w]])
        eng.dma_start(out=dst_ap[p0:p0 + kc * c1c, :], in_=src)


def build_nc():
    nc = bacc.Bacc("TRN2", target_bir_lowering=False, debug=False,
                   num_devices=N_CORES)

    xT_d = nc.dram_tensor("xT", [LENS[0], BATCH], BF, kind="ExternalInput").ap()
    hb = [nc.dram_tensor(f"hb{l}", [PLANS[l]["m_ext"]], BF,
                         kind="ExternalInput").ap() for l in range(3)]
    h1c = [nc.dram_tensor(f"h1c{b}", [512, NB], BF).ap() for b in range(BT)]
    h1f = [nc.dram_tensor(f"h1f{b}", [4096, NB], BF, addr_space="Shared").ap()
           for b in range(BT)]
    h2c = [nc.dram_tensor(f"h2c{b}", [512, NB], BF).ap() for b in range(BT)]
    h2f = [nc.dram_tensor(f"h2f{b}", [4096, NB], BF, addr_space="Shared").ap()
           for b in range(BT)]
    out_d = nc.dram_tensor("outT", [4096, BATCH], BF, kind="ExternalOutput").ap()

    with tile.TileContext(nc) as tc, \
         tc.tile_pool(name="ps", bufs=8, space="PSUM") as psp, \
         tc.tile_pool(name="resid", bufs=1) as residp:

        # h2 stays fully SBUF-resident for L2 (132 KB/partition)
        h2res = [[residp.tile([LT[2][kt][1], NB], BF, name=f"h2r_{b}_{kt}")
                  for kt in range(NKT[2])] for b in range(BT)]

        def load_h2res(b):
            with nc.allow_non_contiguous_dma(reason="perm act load"):
                for kt in range(NKT[2]):
                    _act_dmas(nc, nc.sync, h2f[b].tensor, 2,
                              h2res[b][kt][:], kt, NB, 0, NB)

        # w1sb lives below the L0 pools so its ladder prefetch overlaps L0
        with nc.allow_non_contiguous_dma(reason="hash ladder"), \
             tc.tile_pool(name="l1w", bufs=1) as l1wp:
            w1sb = [l1wp.tile([128, 512], BF, name=f"w1sb{kt}")
                    for kt in range(NKT[1])]

            # ---------------- Layer 0 ----------------
            with tc.tile_pool(name="l0", bufs=1) as l0p, \
                 tc.tile_pool(name="l0x", bufs=18) as l0xp, \
                 tc.tile_pool(name="l0h", bufs=8) as l0hp:
                w0sb = [l0p.tile([LT[0][kt][1], 512], BF, name=f"w0sb{kt}")
                        for kt in range(NKT[0])]
                for kt in range(NKT[0]):
                    _ladder_dmas(nc, nc.scalar, hb[0].tensor, 0, w0sb[kt][:],
                                 kt, 0, 512)
                for kt in range(NKT[1]):
                    _ladder_dmas(nc, nc.scalar, hb[1].tensor, 1, w1sb[kt][:],
                                 kt, 0, 512)

                for b in range(BT):
                    xsb = []
                    for kt in range(NKT[0]):
                        xt = l0xp.tile([LT[0][kt][1], NB], BF, tag="x",
                                       name=f"x_{b}_{kt}")
                        _act_dmas(nc, nc.sync, xT_d.tensor, 0, xt[:], kt,
                                  BATCH, b * NB, NB)
                        xsb.append(xt)
                    for j in range(4):
                        ps = psp.tile([P, NB], F32, tag="ps",
                                      name=f"ps0_{b}_{j}")
                        for kt in range(NKT[0]):
                            nc.tensor.matmul(
                                out=ps[:],
                                lhsT=w0sb[kt][:, j * P:(j + 1) * P],
                                rhs=xsb[kt][:],
                                start=(kt == 0), stop=(kt == NKT[0] - 1))
                        hsb = l0hp.tile([P, NB], BF, tag="h1",
                                        name=f"h1_{b}_{j}")
                        nc.vector.tensor_scalar_max(hsb[:], ps[:], 0.0)
                        nc.sync.dma_start(out=h1c[b][j * P:(j + 1) * P, :],
                                          in_=hsb[:])
                    nc.gpsimd.collective_compute(
                        "AllGather", mybir.AluOpType.bypass, replica_groups=RG,
                        ins=[h1c[b].opt()], outs=[h1f[b].opt()])

            # ---------------- Layer 1 ----------------
            with tc.tile_pool(name="l1r", bufs=6) as l1rp, \
                 tc.tile_pool(name="l1h", bufs=8) as l1hp:
                for b in range(BT):
                    pss = [psp.tile([P, NB], F32, tag="ps",
                                    name=f"ps1_{b}_{j}") for j in range(4)]
                    for kt in range(NKT[1]):
                        rhs = l1rp.tile([P, NB], BF, tag="l1rhs",
                                        name=f"l1r_{b}_{kt}")
                        nc.sync.dma_start(out=rhs[:],
                                          in_=h1f[b][kt * P:(kt + 1) * P, :])
                        for j in range(4):
                            nc.tensor.matmul(
                                out=pss[j][:],
                                lhsT=w1sb[kt][:, j * P:(j + 1) * P],
                                rhs=rhs[:],
                                start=(kt == 0), stop=(kt == NKT[1] - 1))
                    for j in range(4):
                        hsb = l1hp.tile([P, NB], BF, tag="h2",
                                        name=f"h2_{b}_{j}")
                        nc.vector.tensor_scalar_max(hsb[:], pss[j][:], 0.0)
                        nc.sync.dma_start(out=h2c[b][j * P:(j + 1) * P, :],
                                          in_=hsb[:])
                    nc.gpsimd.collective_compute(
                        "AllGather", mybir.AluOpType.bypass, replica_groups=RG,
                        ins=[h2c[b].opt()], outs=[h2f[b].opt()])
                    # residency loads for AGs that finished 2 iterations ago
                    if b >= 2:
                        load_h2res(b - 2)

        # remaining h2 residency loads (AGs already in flight)
        load_h2res(2)
        load_h2res(3)

        # ---------------- Layer 2 ----------------
        with nc.allow_non_contiguous_dma(reason="hash ladder"), \
             tc.tile_pool(name="w2", bufs=56) as w2p, \
             tc.tile_pool(name="l2o", bufs=4) as l2op:
            for jg in range(NJG2):
                slab = []
                for kt in range(NKT[2]):
                    wt = w2p.tile([LT[2][kt][1], 512], BF, tag="w2t",
                                  name=f"w2_{jg}_{kt}")
                    _ladder_dmas(nc, nc.scalar, hb[2].tensor, 2, wt[:],
                                 kt, jg * 512, 512)
                    slab.append(wt)
                for b in range(BT):
                    pss = [psp.tile([P, NB], F32, tag="ps",
                                    name=f"ps2_{jg}_{b}_{j}") for j in range(4)]
                    for kt in range(NKT[2]):
                        for j in range(4):
                            nc.tensor.matmul(
                                out=pss[j][:],
                                lhsT=slab[kt][:, j * P:(j + 1) * P],
                                rhs=h2res[b][kt][:],
                                start=(kt == 0), stop=(kt == NKT[2] - 1))
                    for j in range(4):
                        osb = l2op.tile([P, NB], BF, tag="o",
                                        name=f"o_{jg}_{b}_{j}")
                        nc.vector.tensor_copy(out=osb[:], in_=pss[j][:])
                        nc.sync.dma_start(
                            out=out_d[jg * 512 + j * P:jg * 512 + (j + 1) * P,
                                      b * NB:(b + 1) * NB],
                            in_=osb[:])

    nc.compile()
    return nc


_NC_CACHE = None


def _get_nc():
    global _NC_CACHE
    if _NC_CACHE is None:
        _NC_CACHE = build_nc()
    return _NC_CACHE


def _prep_inputs(x, hw0, hw1, hw2):
    """Host prep: transpose x, build per-core periodic permuted-table slices."""
    x = np.asarray(x, np.float32)
    hws = [np.asarray(hw0, np.float32), np.asarray(hw1, np.float32),
           np.asarray(hw2, np.float32)]
    xT = np.ascontiguousarray(x.T).astype(ml_dtypes.bfloat16)

    per_core_hb = [[None] * 3 for _ in range(N_CORES)]
    for l in range(3):
        pl = PLANS[l]
        N, b = pl["N"], pl["b"]
        m_ext = pl["m_ext"]
        jw = JW[l]
        t0 = pl["u0"] - pl["shift"]          # core-0 slice start (in t-space)
        span = m_ext + (N_CORES - 1) * jw
        t = t0 + np.arange(span, dtype=np.int64)
        shared = hws[l][(b * t) % N].astype(ml_dtypes.bfloat16)
        for c in range(N_CORES):
            per_core_hb[c][l] = shared[c * jw: c * jw + m_ext]
    in_maps = []
    for c in range(N_CORES):
        in_maps.append({
            "xT": xT,
            "hb0": per_core_hb[c][0],
            "hb1": per_core_hb[c][1],
            "hb2": per_core_hb[c][2],
        })
    return in_maps


def kernel(x, hw0, hw1, hw2, trace=False):
    nc = _get_nc()
    in_maps = _prep_inputs(x, hw0, hw1, hw2)
    res = run_bass_kernel_spmd(nc, in_maps, list(range(N_CORES)), trace=trace)
    outs = [np.asarray(res.results[c]["outT"][:JW[2], :])
            for c in range(N_CORES)]
    full = np.concatenate(outs, axis=0)         # [32000, 2048] bf16
    out = np.ascontiguousarray(full.T).astype(np.float32)
    kernel.last_results = res
    return out


# revision 9
# speedup vs baseline: 1.0900x; 1.0216x over previous
"""Hashed-weight MLP (1024-4096-4096-32000, batch 2048) on 8 TRN2 NeuronCores.

Problem: h = relu(x @ W0); h = relu(h @ W1); out = h @ W2, where each
W_l[i, j] = hw_l[(a_l*i + b_l*j + c_l) % N_l] is a virtual (ROBE-Z hashed)
weight gathered from a small parameter vector.

Approach (column-parallel tensor parallelism on all three layers):
  * Via the host-permuted table hw_bb[t] = hw[(b*t) % N] the virtual weight
    becomes row-contiguous: W[i, col] = hw_bb[shift + q*kk + r*c1 + col] with
    i = k*c1 + kk (q = b^-1 a, r = signed residue of q*k mod N). Weight tiles
    are DMAd STRAIGHT from the per-core slice into SBUF (no DRAM
    materialization): one 3-level strided DMA per block-aligned tile for
    L0/L2 (partitions in kk-outer permuted order, with the matching
    permutation applied to the rhs activation tiles), ~4 run DMAs per
    natural-order tile for L1.
  * Each core owns a 1/8 column shard of every layer; shard offsets are
    absorbed into the host-side slice (SPMD-uniform device program).
  * L2 keeps the whole 4096 x 2048 h2 activation resident in SBUF (loaded
    once after each AllGather chunk) and streams W2 tiles through a ring,
    so the big GEMM phase reads each byte from HBM exactly once.
  * Engines: scalar = weight-ladder DMA triggers only; sync = activation
    streams/stores; vector = ReLU + PSUM evacuation (bf16 cast); gpsimd =
    AllGather triggers. GEMMs are bf16 with fp32 PSUM (rel err ~4e-3).
"""
import sys
if "/opt/trn_rl_repo" not in sys.path:
    sys.path.insert(0, "/opt/trn_rl_repo")

import numpy as np
import ml_dtypes

import concourse.bass as bass
import concourse.bacc as bacc
import concourse.tile as tile
import concourse.mybir as mybir
from concourse.bass_utils import run_bass_kernel_spmd

N_CORES = 8
P = 128
NB = 512                      # batch tile (matmul moving dim)
BATCH = 2048
BT = BATCH // NB              # 4

LENS = [1024, 4096, 4096, 32000]
HASH_A = [9973, 10007, 10039]
HASH_B = [31013, 31019, 31039]
HASH_C = [557, 563, 569]
SIZES = [1048576, 1048576, 4194304]
JW = [512, 512, 4000]         # true per-core output shard width
WTOT = [512, 512, 4096]       # max ladder col offset (L2 incl. jg offsets)
TILE_H = [120, 128, 126]      # k-tile heights (L0/L2 block-aligned to k)

BF = mybir.dt.bfloat16
F32 = mybir.dt.float32


def _plan_layer(l):
    N = SIZES[l]; a, b, ch = HASH_A[l], HASH_B[l], HASH_C[l]
    binv = pow(b, -1, N)
    q = (binv * a) % N
    u0 = (binv * ch) % N
    in_dim = LENS[l]
    best = None
    for k in range(1, min(in_dim, 600) + 1):
        r = (q * k) % N
        if r > N // 2:
            r -= N
        C1 = -(-in_dim // k)
        extra = q * (k - 1) + abs(r) * (C1 - 1)
        if best is None or extra < best[0]:
            best = (extra, k, C1, r)
    _, k, C1, r = best
    shift = max(0, -r * (C1 - 1))
    m_ext = shift + q * (k - 1) + max(r, 0) * (C1 - 1) + WTOT[l] + 64
    return dict(N=N, b=b, q=q, u0=u0, k=k, r=r, shift=shift,
                m_ext=m_ext, in_dim=in_dim)


PLANS = [_plan_layer(l) for l in range(3)]
RG = [list(range(N_CORES))]


def _tiles_perm(k, rows_total, tile_h):
    """Block-aligned tiles; seg = (kk0, c1_0, kc, c1c, p0); within a seg the
    DMA streams kk-outer so partition p = p0 + kkrel*c1c + c1rel."""
    tiles = []
    i0 = 0
    while i0 < rows_total:
        h = min(tile_h, rows_total - i0)
        c1_0 = i0 // k
        segs = []
        full_c1 = h // k
        if full_c1:
            segs.append((0, c1_0, k, full_c1, 0))
        rem = h - full_c1 * k
        if rem:
            segs.append((0, c1_0 + full_c1, rem, 1, k * full_c1))
        tiles.append((i0, h, segs))
        i0 += h
    return tiles


def _tiles_nat(k, rows_total, tile_h):
    """Natural-order tiles, <=3 segs each: head run to the block boundary,
    a c1-outer rectangle of full blocks (legal only when r > 0), tail run.
    Natural row order on both the weight and rhs side."""
    tiles = []
    for i0 in range(0, rows_total, tile_h):
        h = min(tile_h, rows_total - i0)
        segs = []
        row = i0
        c1, kk = divmod(row, k)
        if kk:
            cnt = min(k - kk, h)
            segs.append((kk, c1, cnt, 1, row - i0))
            row += cnt
        nfull = (i0 + h - row) // k
        if nfull:
            # c1-outer rect: stream order == natural row order
            segs.append(("rect", row // k, k, nfull, row - i0))
            row += nfull * k
        if row < i0 + h:
            segs.append((0, row // k, i0 + h - row, 1, row - i0))
        tiles.append((i0, h, segs))
    return tiles


LT = [
    _tiles_perm(PLANS[0]["k"], 1024, TILE_H[0]),   # 9 tiles (8x120 + 64)
    _tiles_nat(PLANS[1]["k"], 4096, TILE_H[1]),    # 32 tiles x ~4 segs
    _tiles_perm(PLANS[2]["k"], 4096, TILE_H[2]),   # 33 tiles (32x126 + 64)
]
NKT = [len(t) for t in LT]
NJG2 = 8                      # L2 j-groups of width 512 (4 j-tiles of 128)


def _ladder_dmas(nc, eng, hb_t, l, wtile_ap, tile_idx, col0, w):
    """Emit ladder DMAs for weight tile tile_idx of layer l into SBUF tile."""
    pl = PLANS[l]
    q, r, shift = pl["q"], pl["r"], pl["shift"]
    _, h, segs = LT[l][tile_idx]
    for (kk0, c1_0, kc, c1c, p0) in segs:
        if kk0 == "rect":
            # natural-order full-block rect (c1 outer); requires r > 0
            src = bass.AP(hb_t, shift + r * c1_0 + col0,
                          [[r, c1c], [q, kc], [1, w]])
        elif c1c == 1:
            src = bass.AP(hb_t, shift + q * kk0 + r * c1_0 + col0,
                          [[q, kc], [1, w]])
        else:
            src = bass.AP(hb_t, shift + q * kk0 + r * c1_0 + col0,
                          [[q, kc], [r, c1c], [1, w]])
        eng.dma_start(out=wtile_ap[p0:p0 + kc * c1c, :], in_=src)


def _act_dmas(nc, eng, act_t, l, dst_ap, tile_idx, rowstride, col0, w):
    """Load activation rows matching weight tile tile_idx's partition order.
    act_t is a DRAM tensor whose rows are the layer's contraction rows."""
    pl = PLANS[l]
    k = pl["k"]
    _, h, segs = LT[l][tile_idx]
    for (kk0, c1_0, kc, c1c, p0) in segs:
        base = (k * c1_0 + kk0) * rowstride + col0
        if c1c == 1:
            src = bass.AP(act_t, base, [[rowstride, kc], [1, w]])
        else:
            src = bass.AP(act_t, base,
                          [[rowstride, kc], [k * rowstride, c1c], [1, w]])
        eng.dma_start(out=dst_ap[p0:p0 + kc * c1c, :], in_=src)


def build_nc():
    nc = bacc.Bacc("TRN2", target_bir_lowering=False, debug=False,
                   num_devices=N_CORES)

    xT_d = nc.dram_tensor("xT", [LENS[0], BATCH], BF, kind="ExternalInput").ap()
    hb = [nc.dram_tensor(f"hb{l}", [PLANS[l]["m_ext"]], BF,
                         kind="ExternalInput").ap() for l in range(3)]
    h1c = [nc.dram_tensor(f"h1c{b}", [512, NB], BF).ap() for b in range(BT)]
    h1f = [nc.dram_tensor(f"h1f{b}", [4096, NB], BF, addr_space="Shared").ap()
           for b in range(BT)]
    h2c = [nc.dram_tensor(f"h2c{b}", [512, NB], BF).ap() for b in range(BT)]
    h2f = [nc.dram_tensor(f"h2f{b}", [4096, NB], BF, addr_space="Shared").ap()
           for b in range(BT)]
    out_d = nc.dram_tensor("outT", [4096, BATCH], BF, kind="ExternalOutput").ap()

    with tile.TileContext(nc) as tc, \
         tc.tile_pool(name="ps", bufs=8, space="PSUM") as psp, \
         tc.tile_pool(name="resid", bufs=1) as residp:

        # h2 stays fully SBUF-resident for L2 (132 KB/partition)
        h2res = [[residp.tile([LT[2][kt][1], NB], BF, name=f"h2r_{b}_{kt}")
                  for kt in range(NKT[2])] for b in range(BT)]

        def load_h2res(b):
            with nc.allow_non_contiguous_dma(reason="perm act load"):
                for kt in range(NKT[2]):
                    _act_dmas(nc, nc.sync, h2f[b].tensor, 2,
                              h2res[b][kt][:], kt, NB, 0, NB)

        # head of the first L2 slab: own (outer) pool so its ladders run
        # during L0/L1 without write-after-read waits on dying pools
        NHEAD = 12
        with nc.allow_non_contiguous_dma(reason="hash ladder"), \
             tc.tile_pool(name="jg0h", bufs=1) as jg0hp:
            jg0head = [jg0hp.tile([LT[2][kt][1], 512], BF, name=f"w2h{kt}")
                       for kt in range(NHEAD)]

            # w1sb lives below the L0 pools; its ladders prefetch during L0
            with tc.tile_pool(name="l1w", bufs=1) as l1wp:
                w1sb = [l1wp.tile([128, 512], BF, name=f"w1sb{kt}")
                        for kt in range(NKT[1])]

                # ---------------- Layer 0 ----------------
                with tc.tile_pool(name="l0", bufs=1) as l0p, \
                     tc.tile_pool(name="l0x", bufs=12) as l0xp, \
                     tc.tile_pool(name="l0h", bufs=8) as l0hp:
                    w0sb = [l0p.tile([LT[0][kt][1], 512], BF,
                                     name=f"w0sb{kt}") for kt in range(NKT[0])]
                    for kt in range(NKT[0]):
                        _ladder_dmas(nc, nc.scalar, hb[0].tensor, 0,
                                     w0sb[kt][:], kt, 0, 512)
                    # W1 ladders split across both HWDGE rings
                    for kt in range(0, NKT[1], 2):
                        _ladder_dmas(nc, nc.scalar, hb[1].tensor, 1,
                                     w1sb[kt][:], kt, 0, 512)

                    def load_x(b):
                        xsb = []
                        for kt in range(NKT[0]):
                            xt = l0xp.tile([LT[0][kt][1], NB], BF, tag="x",
                                           name=f"x_{b}_{kt}")
                            _act_dmas(nc, nc.sync, xT_d.tensor, 0, xt[:], kt,
                                      BATCH, b * NB, NB)
                            xsb.append(xt)
                        return xsb

                    xcur = load_x(0)
                    for kt in range(1, NKT[1], 2):
                        _ladder_dmas(nc, nc.sync, hb[1].tensor, 1,
                                     w1sb[kt][:], kt, 0, 512)
                    # first L2 slab head ladders follow W0/W1 on scalar
                    for kt in range(NHEAD):
                        _ladder_dmas(nc, nc.scalar, hb[2].tensor, 2,
                                     jg0head[kt][:], kt, 0, 512)
                    for b in range(BT):
                        xnxt = load_x(b + 1) if b + 1 < BT else None
                        for j in range(4):
                            ps = psp.tile([P, NB], F32, tag="ps",
                                          name=f"ps0_{b}_{j}")
                            for kt in range(NKT[0]):
                                nc.tensor.matmul(
                                    out=ps[:],
                                    lhsT=w0sb[kt][:, j * P:(j + 1) * P],
                                    rhs=xcur[kt][:],
                                    start=(kt == 0), stop=(kt == NKT[0] - 1))
                            hsb = l0hp.tile([P, NB], BF, tag="h1",
                                            name=f"h1_{b}_{j}")
                            nc.vector.tensor_scalar_max(hsb[:], ps[:], 0.0)
                            nc.gpsimd.dma_start(
                                out=h1c[b][j * P:(j + 1) * P, :], in_=hsb[:])
                        nc.gpsimd.collective_compute(
                            "AllGather", mybir.AluOpType.bypass,
                            replica_groups=RG,
                            ins=[h1c[b].opt()], outs=[h1f[b].opt()])
                        xcur = xnxt

                # ---------------- Layer 1 ----------------
                with tc.tile_pool(name="l1r", bufs=10) as l1rp, \
                     tc.tile_pool(name="l1h", bufs=8) as l1hp:
                    for b in range(BT):
                        pss = [psp.tile([P, NB], F32, tag="ps",
                                        name=f"ps1_{b}_{j}")
                               for j in range(4)]
                        for kt in range(NKT[1]):
                            rhs = l1rp.tile([P, NB], BF, tag="l1rhs",
                                            name=f"l1r_{b}_{kt}")
                            nc.sync.dma_start(
                                out=rhs[:],
                                in_=h1f[b][kt * P:(kt + 1) * P, :])
                            for j in range(4):
                                nc.tensor.matmul(
                                    out=pss[j][:],
                                    lhsT=w1sb[kt][:, j * P:(j + 1) * P],
                                    rhs=rhs[:],
                                    start=(kt == 0), stop=(kt == NKT[1] - 1))
                        for j in range(4):
                            hsb = l1hp.tile([P, NB], BF, tag="h2",
                                            name=f"h2_{b}_{j}")
                            nc.vector.tensor_scalar_max(hsb[:], pss[j][:], 0.0)
                            nc.gpsimd.dma_start(
                                out=h2c[b][j * P:(j + 1) * P, :], in_=hsb[:])
                        nc.gpsimd.collective_compute(
                            "AllGather", mybir.AluOpType.bypass,
                            replica_groups=RG,
                            ins=[h2c[b].opt()], outs=[h2f[b].opt()])
                        # residency loads for AGs finished 2 iterations ago
                        if b >= 2:
                            load_h2res(b - 2)

                    # remaining h2 residency loads (AGs already in flight)
                    load_h2res(2)
                    load_h2res(3)

            # ---------------- Layer 2 ----------------
            with tc.tile_pool(name="w2", bufs=56) as w2p, \
                 tc.tile_pool(name="l2o", bufs=4) as l2op:
                for jg in range(NJG2):
                    slab = []
                    for kt in range(NKT[2]):
                        if jg == 0 and kt < NHEAD:
                            slab.append(jg0head[kt])
                            continue
                        wt = w2p.tile([LT[2][kt][1], 512], BF, tag="w2t",
                                      name=f"w2_{jg}_{kt}")
                        _ladder_dmas(nc, nc.scalar, hb[2].tensor, 2, wt[:],
                                     kt, jg * 512, 512)
                        slab.append(wt)
                    for b in range(BT):
                        pss = [psp.tile([P, NB], F32, tag="ps",
                                        name=f"ps2_{jg}_{b}_{j}")
                               for j in range(4)]
                        for kt in range(NKT[2]):
                            for j in range(4):
                                nc.tensor.matmul(
                                    out=pss[j][:],
                                    lhsT=slab[kt][:, j * P:(j + 1) * P],
                                    rhs=h2res[b][kt][:],
                                    start=(kt == 0), stop=(kt == NKT[2] - 1))
                        for j in range(4):
                            osb = l2op.tile([P, NB], BF, tag="o",
                                            name=f"o_{jg}_{b}_{j}")
                            nc.vector.tensor_copy(out=osb[:], in_=pss[j][:])
                            nc.sync.dma_start(
                                out=out_d[jg * 512 + j * P:
                                          jg * 512 + (j + 1) * P,
                                          b * NB:(b + 1) * NB],
                                in_=osb[:])

    nc.compile()
    return nc


_NC_CACHE = None


def _get_nc():
    global _NC_CACHE
    if _NC_CACHE is None:
        _NC_CACHE = build_nc()
    return _NC_CACHE


def _prep_inputs(x, hw0, hw1, hw2):
    """Host prep: transpose x, build per-core periodic permuted-table slices."""
    x = np.asarray(x, np.float32)
    hws = [np.asarray(hw0, np.float32), np.asarray(hw1, np.float32),
           np.asarray(hw2, np.float32)]
    xT = np.ascontiguousarray(x.T).astype(ml_dtypes.bfloat16)

    per_core_hb = [[None] * 3 for _ in range(N_CORES)]
    for l in range(3):
        pl = PLANS[l]
        N, b = pl["N"], pl["b"]
        m_ext = pl["m_ext"]
        jw = JW[l]
        t0 = pl["u0"] - pl["shift"]          # core-0 slice start (in t-space)
        span = m_ext + (N_CORES - 1) * jw
        t = t0 + np.arange(span, dtype=np.int64)
        shared = hws[l][(b * t) % N].astype(ml_dtypes.bfloat16)
        for c in range(N_CORES):
            per_core_hb[c][l] = shared[c * jw: c * jw + m_ext]
    in_maps = []
    for c in range(N_CORES):
        in_maps.append({
            "xT": xT,
            "hb0": per_core_hb[c][0],
            "hb1": per_core_hb[c][1],
            "hb2": per_core_hb[c][2],
        })
    return in_maps


def kernel(x, hw0, hw1, hw2, trace=False):
    nc = _get_nc()
    in_maps = _prep_inputs(x, hw0, hw1, hw2)
    res = run_bass_kernel_spmd(nc, in_maps, list(range(N_CORES)), trace=trace)
    outs = [np.asarray(res.results[c]["outT"][:JW[2], :])
            for c in range(N_CORES)]
    full = np.concatenate(outs, axis=0)         # [32000, 2048] bf16
    out = np.ascontiguousarray(full.T).astype(np.float32)
    kernel.last_results = res
    return out


# revision 10
# speedup vs baseline: 1.1344x; 1.0408x over previous
"""Hashed-weight MLP (1024-4096-4096-32000, batch 2048) on 8 TRN2 NeuronCores.

Problem: h = relu(x @ W0); h = relu(h @ W1); out = h @ W2, where each
W_l[i, j] = hw_l[(a_l*i + b_l*j + c_l) % N_l] is a virtual (ROBE-Z hashed)
weight gathered from a small parameter vector.

Approach (column-parallel tensor parallelism on all three layers):
  * Via the host-permuted table hw_bb[t] = hw[(b*t) % N] the virtual weight
    becomes row-contiguous: W[i, col] = hw_bb[shift + q*kk + r*c1 + col] with
    i = k*c1 + kk (q = b^-1 a, r = signed residue of q*k mod N). Weight tiles
    are DMAd STRAIGHT from the per-core slice into SBUF (no DRAM
    materialization): one 3-level strided DMA per block-aligned tile for
    L0/L2 (partitions in kk-outer permuted order, with the matching
    permutation applied to the rhs activation tiles), ~4 run DMAs per
    natural-order tile for L1.
  * Each core owns a 1/8 column shard of every layer; shard offsets are
    absorbed into the host-side slice (SPMD-uniform device program).
  * L2 keeps the whole 4096 x 2048 h2 activation resident in SBUF (loaded
    once after each AllGather chunk) and streams W2 tiles through a ring,
    so the big GEMM phase reads each byte from HBM exactly once.
  * Engines: scalar = weight-ladder DMA triggers only; sync = activation
    streams/stores; vector = ReLU + PSUM evacuation (bf16 cast); gpsimd =
    AllGather triggers. GEMMs are bf16 with fp32 PSUM (rel err ~4e-3).
"""
import sys
if "/opt/trn_rl_repo" not in sys.path:
    sys.path.insert(0, "/opt/trn_rl_repo")

import numpy as np
import ml_dtypes

import concourse.bass as bass
import concourse.bacc as bacc
import concourse.tile as tile
import concourse.mybir as mybir
from concourse.bass_utils import run_bass_kernel_spmd

N_CORES = 8
P = 128
NB = 512                      # batch tile (matmul moving dim)
BATCH = 2048
BT = BATCH // NB              # 4

LENS = [1024, 4096, 4096, 32000]
HASH_A = [9973, 10007, 10039]
HASH_B = [31013, 31019, 31039]
HASH_C = [557, 563, 569]
SIZES = [1048576, 1048576, 4194304]
JW = [512, 512, 4000]         # true per-core output shard width
WTOT = [512, 512, 4096]       # max ladder col offset (L2 incl. jg offsets)
TILE_H = [120, 128, 126]      # k-tile heights (L0/L2 block-aligned to k)

BF = mybir.dt.bfloat16
F32 = mybir.dt.float32


def _plan_layer(l):
    N = SIZES[l]; a, b, ch = HASH_A[l], HASH_B[l], HASH_C[l]
    binv = pow(b, -1, N)
    q = (binv * a) % N
    u0 = (binv * ch) % N
    in_dim = LENS[l]
    best = None
    for k in range(1, min(in_dim, 600) + 1):
        r = (q * k) % N
        if r > N // 2:
            r -= N
        C1 = -(-in_dim // k)
        extra = q * (k - 1) + abs(r) * (C1 - 1)
        if best is None or extra < best[0]:
            best = (extra, k, C1, r)
    _, k, C1, r = best
    shift = max(0, -r * (C1 - 1))
    m_ext = shift + q * (k - 1) + max(r, 0) * (C1 - 1) + WTOT[l] + 64
    return dict(N=N, b=b, q=q, u0=u0, k=k, r=r, shift=shift,
                m_ext=m_ext, in_dim=in_dim)


PLANS = [_plan_layer(l) for l in range(3)]
RG = [list(range(N_CORES))]


def _tiles_perm(k, rows_total, tile_h):
    """Block-aligned tiles; seg = (kk0, c1_0, kc, c1c, p0); within a seg the
    DMA streams kk-outer so partition p = p0 + kkrel*c1c + c1rel."""
    tiles = []
    i0 = 0
    while i0 < rows_total:
        h = min(tile_h, rows_total - i0)
        c1_0 = i0 // k
        segs = []
        full_c1 = h // k
        if full_c1:
            segs.append((0, c1_0, k, full_c1, 0))
        rem = h - full_c1 * k
        if rem:
            segs.append((0, c1_0 + full_c1, rem, 1, k * full_c1))
        tiles.append((i0, h, segs))
        i0 += h
    return tiles


def _tiles_nat(k, rows_total, tile_h):
    """Natural-order tiles, <=3 segs each: head run to the block boundary,
    a c1-outer rectangle of full blocks (legal only when r > 0), tail run.
    Natural row order on both the weight and rhs side."""
    tiles = []
    for i0 in range(0, rows_total, tile_h):
        h = min(tile_h, rows_total - i0)
        segs = []
        row = i0
        c1, kk = divmod(row, k)
        if kk:
            cnt = min(k - kk, h)
            segs.append((kk, c1, cnt, 1, row - i0))
            row += cnt
        nfull = (i0 + h - row) // k
        if nfull:
            # c1-outer rect: stream order == natural row order
            segs.append(("rect", row // k, k, nfull, row - i0))
            row += nfull * k
        if row < i0 + h:
            segs.append((0, row // k, i0 + h - row, 1, row - i0))
        tiles.append((i0, h, segs))
    return tiles


LT = [
    _tiles_perm(PLANS[0]["k"], 1024, TILE_H[0]),   # 9 tiles (8x120 + 64)
    _tiles_nat(PLANS[1]["k"], 4096, TILE_H[1]),    # 32 tiles x ~4 segs
    _tiles_perm(PLANS[2]["k"], 4096, TILE_H[2]),   # 33 tiles (32x126 + 64)
]
NKT = [len(t) for t in LT]
NJG2 = 8                      # L2 j-groups of width 512 (4 j-tiles of 128)


def _ladder_dmas(nc, eng, hb_t, l, wtile_ap, tile_idx, col0, w):
    """Emit ladder DMAs for weight tile tile_idx of layer l into SBUF tile."""
    pl = PLANS[l]
    q, r, shift = pl["q"], pl["r"], pl["shift"]
    _, h, segs = LT[l][tile_idx]
    for (kk0, c1_0, kc, c1c, p0) in segs:
        if kk0 == "rect":
            # natural-order full-block rect (c1 outer); requires r > 0
            src = bass.AP(hb_t, shift + r * c1_0 + col0,
                          [[r, c1c], [q, kc], [1, w]])
        elif c1c == 1:
            src = bass.AP(hb_t, shift + q * kk0 + r * c1_0 + col0,
                          [[q, kc], [1, w]])
        else:
            src = bass.AP(hb_t, shift + q * kk0 + r * c1_0 + col0,
                          [[q, kc], [r, c1c], [1, w]])
        eng.dma_start(out=wtile_ap[p0:p0 + kc * c1c, :], in_=src)


def _act_dmas(nc, eng, act_t, l, dst_ap, tile_idx, rowstride, col0, w):
    """Load activation rows matching weight tile tile_idx's partition order.
    act_t is a DRAM tensor whose rows are the layer's contraction rows."""
    pl = PLANS[l]
    k = pl["k"]
    _, h, segs = LT[l][tile_idx]
    for (kk0, c1_0, kc, c1c, p0) in segs:
        base = (k * c1_0 + kk0) * rowstride + col0
        if c1c == 1:
            src = bass.AP(act_t, base, [[rowstride, kc], [1, w]])
        else:
            src = bass.AP(act_t, base,
                          [[rowstride, kc], [k * rowstride, c1c], [1, w]])
        eng.dma_start(out=dst_ap[p0:p0 + kc * c1c, :], in_=src)


def build_nc():
    nc = bacc.Bacc("TRN2", target_bir_lowering=False, debug=False,
                   num_devices=N_CORES)

    xT_d = nc.dram_tensor("xT", [LENS[0], BATCH], BF, kind="ExternalInput").ap()
    hb = [nc.dram_tensor(f"hb{l}", [PLANS[l]["m_ext"]], BF,
                         kind="ExternalInput").ap() for l in range(3)]
    # batch-pair activation chunks (1024-wide AllGathers: 2KB DMA runs)
    h1c = [nc.dram_tensor(f"h1c{p}", [512, 2 * NB], BF).ap() for p in range(2)]
    h1f = [nc.dram_tensor(f"h1f{p}", [4096, 2 * NB], BF,
                          addr_space="Shared").ap() for p in range(2)]
    h2c = [nc.dram_tensor(f"h2c{p}", [512, 2 * NB], BF).ap() for p in range(2)]
    h2f = [nc.dram_tensor(f"h2f{p}", [4096, 2 * NB], BF,
                          addr_space="Shared").ap() for p in range(2)]
    out_d = nc.dram_tensor("outT", [4096, BATCH], BF, kind="ExternalOutput").ap()

    with tile.TileContext(nc) as tc, \
         tc.tile_pool(name="ps", bufs=8, space="PSUM") as psp, \
         tc.tile_pool(name="resid", bufs=1) as residp:

        # h2 stays fully SBUF-resident for L2 (132 KB/partition)
        h2res = [[residp.tile([LT[2][kt][1], 2 * NB], BF, name=f"h2r_{p}_{kt}")
                  for kt in range(NKT[2])] for p in range(2)]

        def load_h2res(p):
            # on gpsimd (SWDGE): third DGE lane, keeps both HWDGE rings free
            with nc.allow_non_contiguous_dma(reason="perm act load"):
                for kt in range(NKT[2]):
                    _act_dmas(nc, nc.gpsimd, h2f[p].tensor, 2,
                              h2res[p][kt][:], kt, 2 * NB, 0, 2 * NB)

        # head of the first L2 slab: own (outer) pool so its ladders run
        # during L0/L1 without write-after-read waits on dying pools
        NHEAD = 12
        with nc.allow_non_contiguous_dma(reason="hash ladder"), \
             tc.tile_pool(name="jg0h", bufs=1) as jg0hp:
            jg0head = [jg0hp.tile([LT[2][kt][1], 512], BF, name=f"w2h{kt}")
                       for kt in range(NHEAD)]

            # w1sb lives below the L0 pools; its ladders prefetch during L0
            with tc.tile_pool(name="l1w", bufs=1) as l1wp:
                w1sb = [l1wp.tile([128, 512], BF, name=f"w1sb{kt}")
                        for kt in range(NKT[1])]

                # ---------------- Layer 0 ----------------
                with tc.tile_pool(name="l0", bufs=1) as l0p, \
                     tc.tile_pool(name="l0x", bufs=12) as l0xp, \
                     tc.tile_pool(name="l0h", bufs=8) as l0hp:
                    w0sb = [l0p.tile([LT[0][kt][1], 512], BF,
                                     name=f"w0sb{kt}") for kt in range(NKT[0])]
                    for kt in range(NKT[0]):
                        _ladder_dmas(nc, nc.scalar, hb[0].tensor, 0,
                                     w0sb[kt][:], kt, 0, 512)
                    for kt in range(NKT[1]):
                        _ladder_dmas(nc, nc.scalar, hb[1].tensor, 1,
                                     w1sb[kt][:], kt, 0, 512)
                    for kt in range(NHEAD):
                        _ladder_dmas(nc, nc.scalar, hb[2].tensor, 2,
                                     jg0head[kt][:], kt, 0, 512)

                    def load_x(b):
                        xsb = []
                        for kt in range(NKT[0]):
                            xt = l0xp.tile([LT[0][kt][1], NB], BF, tag="x",
                                           name=f"x_{b}_{kt}")
                            _act_dmas(nc, nc.sync, xT_d.tensor, 0, xt[:], kt,
                                      BATCH, b * NB, NB)
                            xsb.append(xt)
                        return xsb

                    xcur = load_x(0)
                    for b in range(BT):
                        p, hf = divmod(b, 2)
                        xnxt = load_x(b + 1) if b + 1 < BT else None
                        for j in range(4):
                            ps = psp.tile([P, NB], F32, tag="ps",
                                          name=f"ps0_{b}_{j}")
                            for kt in range(NKT[0]):
                                nc.tensor.matmul(
                                    out=ps[:],
                                    lhsT=w0sb[kt][:, j * P:(j + 1) * P],
                                    rhs=xcur[kt][:],
                                    start=(kt == 0), stop=(kt == NKT[0] - 1))
                            hsb = l0hp.tile([P, NB], BF, tag="h1",
                                            name=f"h1_{b}_{j}")
                            nc.vector.tensor_scalar_max(hsb[:], ps[:], 0.0)
                            nc.gpsimd.dma_start(
                                out=h1c[p][j * P:(j + 1) * P,
                                           hf * NB:(hf + 1) * NB], in_=hsb[:])
                        if hf == 1:
                            nc.gpsimd.collective_compute(
                                "AllGather", mybir.AluOpType.bypass,
                                replica_groups=RG,
                                ins=[h1c[p].opt()], outs=[h1f[p].opt()])
                        xcur = xnxt

                # ---------------- Layer 1 (pair sweeps) ----------------
                with tc.tile_pool(name="l1r", bufs=6) as l1rp, \
                     tc.tile_pool(name="l1h", bufs=8) as l1hp:
                    for p in range(2):
                        pss = [psp.tile([P, NB], F32, tag="ps",
                                        name=f"ps1_{p}_{u}") for u in range(8)]
                        for kt in range(NKT[1]):
                            rhs = l1rp.tile([P, 2 * NB], BF, tag="l1rhs",
                                            name=f"l1r_{p}_{kt}")
                            nc.sync.dma_start(
                                out=rhs[:],
                                in_=h1f[p][kt * P:(kt + 1) * P, :])
                            for j in range(4):
                                for hf in range(2):
                                    nc.tensor.matmul(
                                        out=pss[j * 2 + hf][:],
                                        lhsT=w1sb[kt][:, j * P:(j + 1) * P],
                                        rhs=rhs[:, hf * NB:(hf + 1) * NB],
                                        start=(kt == 0),
                                        stop=(kt == NKT[1] - 1))
                        for j in range(4):
                            for hf in range(2):
                                hsb = l1hp.tile([P, NB], BF, tag="h2",
                                                name=f"h2_{p}_{j}_{hf}")
                                nc.vector.tensor_scalar_max(
                                    hsb[:], pss[j * 2 + hf][:], 0.0)
                                nc.gpsimd.dma_start(
                                    out=h2c[p][j * P:(j + 1) * P,
                                               hf * NB:(hf + 1) * NB],
                                    in_=hsb[:])
                        nc.gpsimd.collective_compute(
                            "AllGather", mybir.AluOpType.bypass,
                            replica_groups=RG,
                            ins=[h2c[p].opt()], outs=[h2f[p].opt()])
                        load_h2res(p)

            # ---------------- Layer 2 ----------------
            with tc.tile_pool(name="w2", bufs=56) as w2p, \
                 tc.tile_pool(name="l2o", bufs=4) as l2op:
                for jg in range(NJG2):
                    slab = []
                    for kt in range(NKT[2]):
                        if jg == 0 and kt < NHEAD:
                            slab.append(jg0head[kt])
                            continue
                        wt = w2p.tile([LT[2][kt][1], 512], BF, tag="w2t",
                                      name=f"w2_{jg}_{kt}")
                        _ladder_dmas(nc, nc.scalar, hb[2].tensor, 2, wt[:],
                                     kt, jg * 512, 512)
                        slab.append(wt)
                    for b in range(BT):
                        p, hf = divmod(b, 2)
                        pss = [psp.tile([P, NB], F32, tag="ps",
                                        name=f"ps2_{jg}_{b}_{j}")
                               for j in range(4)]
                        for kt in range(NKT[2]):
                            for j in range(4):
                                nc.tensor.matmul(
                                    out=pss[j][:],
                                    lhsT=slab[kt][:, j * P:(j + 1) * P],
                                    rhs=h2res[p][kt][:, hf * NB:(hf + 1) * NB],
                                    start=(kt == 0), stop=(kt == NKT[2] - 1))
                        for j in range(4):
                            osb = l2op.tile([P, NB], BF, tag="o",
                                            name=f"o_{jg}_{b}_{j}")
                            nc.vector.tensor_copy(out=osb[:], in_=pss[j][:])
                            nc.sync.dma_start(
                                out=out_d[jg * 512 + j * P:
                                          jg * 512 + (j + 1) * P,
                                          b * NB:(b + 1) * NB],
                                in_=osb[:])

    nc.compile()
    return nc


_NC_CACHE = None


def _get_nc():
    global _NC_CACHE
    if _NC_CACHE is None:
        _NC_CACHE = build_nc()
    return _NC_CACHE


def _prep_inputs(x, hw0, hw1, hw2):
    """Host prep: transpose x, build per-core periodic permuted-table slices."""
    x = np.asarray(x, np.float32)
    hws = [np.asarray(hw0, np.float32), np.asarray(hw1, np.float32),
           np.asarray(hw2, np.float32)]
    xT = np.ascontiguousarray(x.T).astype(ml_dtypes.bfloat16)

    per_core_hb = [[None] * 3 for _ in range(N_CORES)]
    for l in range(3):
        pl = PLANS[l]
        N, b = pl["N"], pl["b"]
        m_ext = pl["m_ext"]
        jw = JW[l]
        t0 = pl["u0"] - pl["shift"]          # core-0 slice start (in t-space)
        span = m_ext + (N_CORES - 1) * jw
        t = t0 + np.arange(span, dtype=np.int64)
        shared = hws[l][(b * t) % N].astype(ml_dtypes.bfloat16)
        for c in range(N_CORES):
            per_core_hb[c][l] = shared[c * jw: c * jw + m_ext]
    in_maps = []
    for c in range(N_CORES):
        in_maps.append({
            "xT": xT,
            "hb0": per_core_hb[c][0],
            "hb1": per_core_hb[c][1],
            "hb2": per_core_hb[c][2],
        })
    return in_maps


def kernel(x, hw0, hw1, hw2, trace=False):
    nc = _get_nc()
    in_maps = _prep_inputs(x, hw0, hw1, hw2)
    res = run_bass_kernel_spmd(nc, in_maps, list(range(N_CORES)), trace=trace)
    outs = [np.asarray(res.results[c]["outT"][:JW[2], :])
            for c in range(N_CORES)]
    full = np.concatenate(outs, axis=0)         # [32000, 2048] bf16
    out = np.ascontiguousarray(full.T).astype(np.float32)
    kernel.last_results = res
    return out
